# revision 53
# baseline (speedup 1.0000x reference)
"""Sparse-attention SPMD kernel (one NeuronCore program), v7.

Per core: B=2 batches x NH=8 heads as 4*B head-pairs (A, B).

PE model (measured): strictly one moving stream at a time, 1 col/cycle
@2.4GHz warm; LDWEIGHTS hides behind the previous stream. Optimization
= minimize total streamed columns:
  - projections: A/B heads share the SAME moving stream (hx chunk), so
    they are fused into one 128-row matmul (half the streams of v5)
  - S = Q.K^T: 3 bf16 passes (hi*hi + hi*lo + lo*hi ~= 17-bit), A/B are
    row-split 64-contraction streams (cannot fuse: different data)
  - mask add: identity matmul accumulates -1e30 bias into S psum
  - softmax: DVE negmax reduce; ACT exp (bias=negmax) writes bf16 P
    directly into the transpose slab, accum rowsum; NO normalize on
    device - host divides by rowsum (saves DVE mul+recip)
  - transpose: DMA-xbar per 2-qt slab [128, 2048] on sync HWDGE ring
  - AV: P^T is the STATIONARY operand (PT slab [m,q] tiles as weights),
    V streams (64 cols/step instead of 512): AV halves to 4096
    stream-cols per (b,h). Output lands q-major [128(q), qt, d] in one
    psum bank per half-pair, DMA'd straight to DRAM [B, NH, G, D].
Host: out / rowsum (float64 divide), no transpose needed.
"""
import sys

sys.path.insert(0, '/opt/trn_rl_repo')
from contextlib import ExitStack

import concourse.bass as bass
import concourse.tile as tile
from concourse import bacc, mybir

FP32 = mybir.dt.float32
BF16 = mybir.dt.bfloat16
FP16 = mybir.dt.float16
F8E4 = mybir.dt.float8e4
AF = mybir.ActivationFunctionType
ALU = mybir.AluOpType
PM = mybir.MatmulPerfMode


def build_attention(B=2, NH=8, G=1024, I=256, D=64, tp_batch=2):
    assert D == 64 and I % 128 == 0 and G % 512 == 0 and NH % 2 == 0
    KT = I // 128          # contraction k-tiles for projections
    QT = G // 128          # q tiles
    MC = G // 512          # m chunks of 512 (S rhs / psum bank)
    MT = G // 128          # m tiles
    Q2 = QT // 2           # 2-qt transpose slabs

    nc = bacc.Bacc(None, target_bir_lowering=False, debug=False)
    dram = {}
    for nm, shp, dt in [
        ("hTb", [B, I, G], BF16), ("hTl", [B, I, G], BF16),
        ("mbp", [G, G], BF16),
        ("wqh", [I, NH * D], BF16), ("wql", [I, NH * D], BF16),
        ("wkh", [I, NH * D], BF16), ("wkl", [I, NH * D], BF16),
        ("wvp", [KT, 128, NH * D], BF16),
        ("ident", [128, 128], BF16), ("identn", [128, 128], FP16),
    ]:
        dram[nm] = nc.declare_dram_parameter(nm, shp, dt, isOutput=False)
    # device-natural layouts (contiguous per partition; host reorders):
    # out[b, h, half, p, qt, d] with g = half*512 + qt*128 + p
    out_ext = nc.declare_dram_parameter("out", [B, NH, 2, 128, QT // 2, D],
                                        FP32, isOutput=True)
    # rs[b, h, p, qt] with g = qt*128 + p
    rs_ext = nc.declare_dram_parameter("rs", [B, NH, 128, QT], FP32,
                                       isOutput=True)

    ctx = ExitStack()
    with ctx:
        tc = ctx.enter_context(tile.TileContext(nc))
        const = ctx.enter_context(tc.tile_pool(name="const", bufs=1))
        vpool = ctx.enter_context(tc.tile_pool(name="vsb", bufs=1))
        qk_pool = ctx.enter_context(tc.tile_pool(name="qk", bufs=2))
        f8_pool = ctx.enter_context(tc.tile_pool(name="f8", bufs=2))
        pn_pool = ctx.enter_context(tc.tile_pool(name="pn", bufs=6))
        pt_pool = ctx.enter_context(tc.tile_pool(name="pt", bufs=4))
        o_pool = ctx.enter_context(tc.tile_pool(name="o", bufs=4))
        st_pool = ctx.enter_context(tc.tile_pool(name="stats", bufs=4))
        # PSUM budget (8 banks): pss 3x2 + misc 2x1 = 8. The misc pool is
        # shared by proj / V-proj / AV psums (temporally mostly disjoint;
        # a single tag rotates the two banks).
        ps_s = ctx.enter_context(tc.tile_pool(name="pss", bufs=3,
                                              space="PSUM"))
        ps_misc = ctx.enter_context(tc.tile_pool(name="psmisc", bufs=2,
                                                 space="PSUM"))

        # ---------- setup: load inputs (gpsimd SWDGE ring; sync ring is
        # reserved for the xbar transposes) ----------
        # DMA issue order matters: proj(0) needs hT(b0) + wq + identn first;
        # mask (mbp/ident) is only needed ~15us in.
        hTb_sb = const.tile([128, B, KT, G], BF16)
        hTl_sb = const.tile([128, B, KT, G], BF16)
        w_sb = {}
        for nm in ("wqh", "wql", "wkh", "wkl"):
            # [128(i), kt, h, d] heads contiguous within a kt (packed on
            # host): the head-pair weight slice [:, kt, 2hp:2hp+2, :]
            # merges to a single 128-elem free dim
            w_sb[nm] = const.tile([128, KT, NH, D], BF16, name=nm)
        wv_sb = const.tile([128, KT, NH * D], BF16)
        ident_sb = const.tile([128, 128], BF16)
        identn_sb = const.tile([128, 128], FP16)
        mbp_sb = const.tile([128, QT, G], BF16)

        def _load_w(nm):
            nc.gpsimd.dma_start(
                out=w_sb[nm][:],
                in_=dram[nm].rearrange("(kt p) (h d) -> p kt h d",
                                       p=128, d=D))

        def _load_h(nm, sb, b):
            nc.gpsimd.dma_start(
                out=sb[:, b],
                in_=dram[nm][b].rearrange("(kt p) g -> p kt g", p=128))

        def _load_h_kt(nm, sb, b, kt):
            nc.gpsimd.dma_start(
                out=sb[:, b, kt],
                in_=dram[nm][b, 128 * kt:128 * (kt + 1), :])

        _load_w("wqh")
        _load_h_kt("hTb", hTb_sb, 0, 0)
        _load_w("wql")
        _load_h_kt("hTl", hTl_sb, 0, 0)
        _load_h_kt("hTb", hTb_sb, 0, 1)
        _load_h_kt("hTl", hTl_sb, 0, 1)
        nc.gpsimd.dma_start(out=identn_sb[:], in_=dram["identn"][:])
        _load_w("wkh")
        _load_w("wkl")
        nc.gpsimd.dma_start(out=wv_sb[:],
                            in_=dram["wvp"].rearrange("kt p hd -> p kt hd"))
        nc.gpsimd.dma_start(out=ident_sb[:], in_=dram["ident"][:])
        nc.gpsimd.dma_start(
            out=mbp_sb[:],
            in_=dram["mbp"].rearrange("(qt p) m -> p qt m", p=128))
        for b in range(1, B):
            _load_h("hTb", hTb_sb, b)
            _load_h("hTl", hTl_sb, b)

        # V for both batches: [128(m), b, mt, NH*D] bf16
        v_sb = vpool.tile([128, B, MT, NH * D], BF16)

        def emit_v_batch(b):
            for mt in range(MT):
                psv = ps_misc.tile([128, 512], FP32, tag="m",
                                   name=f"psv{b}{mt}")
                for kt in range(KT):
                    nc.tensor.matmul(
                        psv[:], hTb_sb[:, b, kt, 128 * mt:128 * (mt + 1)],
                        wv_sb[:, kt], start=(kt == 0), stop=(kt == KT - 1))
                if mt % 2 == 0:
                    nc.scalar.copy(v_sb[:, b, mt], psv[:])
                else:
                    nc.vector.tensor_copy(v_sb[:, b, mt], psv[:])

        # ---------- main loop over (batch, head-pair), software-pipelined ----
        pairs = [(b, hp) for b in range(B) for hp in range(NH // 2)]
        state = {}

        def emit_proj(i):
            """Q^T/K^T for pair i: 3-pass bf16 projections; A/B heads
            fused into one 128-row matmul per (chunk, kt, pass) since
            they share the moving stream. psum group is then split into
            fp16 hi/lo tiles (ACT copy, -I matmul, DVE copy), and
            scale-balanced fp8e4 copies are derived on gpsimd for the
            DoubleRow S cross pass:
              f8q planes (W): qh*2^-4, ql*2^7; f8k planes (X): kl*2^4,
              kh*2^-7 -> DR computes qh*kl + ql*kh with product scale 1.
            """
            b, hp = pairs[i]
            qk_hi = qk_pool.tile([128, 2 * G], FP16, tag="qkh", name=f"qkh{i}")
            qk_lo = qk_pool.tile([128, 2 * G], FP16, tag="qkl", name=f"qkl{i}")
            f8q = f8_pool.tile([128, 2, G], F8E4, tag="f8q", name=f"f8q{i}")
            f8k = f8_pool.tile([128, 2, G], F8E4, tag="f8k", name=f"f8k{i}")
            for W, off in (("q", 0), ("k", G)):
                wh, wl = w_sb["w" + W + "h"], w_sb["w" + W + "l"]
                for chk in range(G // 512):
                    sl = slice(512 * chk, 512 * (chk + 1))
                    psp = ps_misc.tile([128, 512], FP32, tag="m",
                                       name=f"psp{i}{W}{chk}")
                    first = True
                    for kt in range(KT):
                        for wx, hx in ((wh, hTb_sb), (wh, hTl_sb),
                                       (wl, hTb_sb)):
                            # [128, 2, 64]->[128, 128] weight AP (merged)
                            nc.tensor.matmul(
                                psp[:],
                                wx[:, kt, 2 * hp:2 * hp + 2, :],
                                hx[:, b, kt, sl],
                                start=first, stop=False,
                                skip_group_check=True)
                            first = False
                    csl = slice(off + 512 * chk, off + 512 * (chk + 1))
                    nc.scalar.copy(qk_hi[:, csl], psp[:])
                    # psp -= hi  (exact residual), then lo = fp16(psp)
                    nc.tensor.matmul(psp[:], identn_sb[:], qk_hi[:, csl],
                                     start=False, stop=True,
                                     skip_group_check=True)
                    nc.vector.tensor_copy(qk_lo[:, csl], psp[:])
                    if W == "q":
                        nc.scalar.activation(f8q[:, 0, sl], qk_hi[:, csl],
                                             AF.Copy, scale=2.0 ** -4)
                        nc.vector.tensor_scalar_mul(
                            f8q[:, 1, sl], qk_lo[:, csl], 2.0 ** 7)
                    else:
                        nc.vector.tensor_scalar_mul(
                            f8k[:, 0, sl], qk_lo[:, csl], 2.0 ** 4)
                        nc.scalar.activation(f8k[:, 1, sl], qk_hi[:, csl],
                                             AF.Copy, scale=2.0 ** -7)
            state[i] = (qk_hi, qk_lo, f8q, f8k)

        rows_of = {"A": slice(0, 64), "B": slice(64, 128)}

        emit_proj(0)
        emit_v_batch(0)
        emit_proj(1)
        for i, (b, hp) in enumerate(pairs):
            hA, hB = 2 * hp, 2 * hp + 1
            qk_hi, qk_lo, f8q, f8k = state.pop(i)

            negmax = {X: st_pool.tile([128, QT], FP32, tag=f"negmax{X}",
                                      name=f"negmax{X}{i}") for X in "AB"}
            rowsum = {X: st_pool.tile([128, QT], FP32, tag=f"rowsum{X}",
                                      name=f"rowsum{X}{i}") for X in "AB"}
            # PT layout: [m_in(128), qt2, qt_lo(2), mt(8), q_in(128)]
            PT = {X: pt_pool.tile([128, Q2, 2, MT, 128], BF16, tag="pt",
                                  name=f"PT{X}{i}") for X in "AB"}
            pn2 = {}

            av_ps = {}

            def emit_av_slab(s):
                """AV for slab s (qts 2s, 2s+1): PT tiles as weights, V
                streams 64 cols; accumulate over mt into one psum bank
                per half-pair; after an odd slab, SBUF-stage + DMA out."""
                half = s // 2
                if s % 2 == 0:
                    av_ps[half] = ps_misc.tile([128, QT // 2, 128], FP32,
                                               tag="m", name=f"av{i}{half}")
                av = av_ps[half]
                for l in range(2):
                    qt = 2 * s + l
                    lq = qt % (QT // 2)
                    for X, hh in (("A", hA), ("B", hB)):
                        xo = 0 if X == "A" else 64
                        for mt in range(MT):
                            nc.tensor.matmul(
                                av[:, lq, xo:xo + 64],
                                PT[X][:, s, l, mt, :],
                                v_sb[:, b, mt, 64 * hh:64 * (hh + 1)],
                                start=(mt == 0), stop=(mt == MT - 1),
                                skip_group_check=True)
                if s % 2 == 1:
                    o_sb = o_pool.tile([128, QT // 2, 128], FP32, tag="o",
                                       name=f"o{i}{half}")
                    if half == 0:
                        nc.scalar.copy(o_sb[:], av[:])
                    else:
                        nc.vector.tensor_copy(o_sb[:], av[:])
                    for X, hh in (("A", hA), ("B", hB)):
                        xo = 0 if X == "A" else 64
                        nc.gpsimd.dma_start(
                            out=out_ext[b, hh, half],
                            in_=o_sb[:, :, xo:xo + 64])

            for qt in range(QT):
                pss = {X: ps_s.tile([128, G], FP32, tag="s",
                                    name=f"pss{X}{i}{qt}") for X in "AB"}
                # S: 3 bf16 passes per (qt, chunk, X); A/B row groups
                qsl = slice(128 * qt, 128 * (qt + 1))
                for chk in range(MC):
                    sl = slice(512 * chk, 512 * (chk + 1))
                    ksl = slice(G + 512 * chk, G + 512 * (chk + 1))
                    for X in "AB":
                        rows = rows_of[X]
                        # pass 1: hi*hi in fp16
                        nc.tensor.matmul(
                            pss[X][:, sl], qk_hi[rows, qsl],
                            qk_hi[rows, ksl],
                            start=True, stop=False, skip_group_check=True)
                        # pass 2: both cross terms in one fp8 DoubleRow
                        nc.tensor.matmul(
                            pss[X][:, sl], f8q[rows, :, qsl],
                            f8k[rows, :, sl],
                            perf_mode=PM.DoubleRow,
                            start=False, stop=False, skip_group_check=True)
                # mask add via identity matmul (PE), both heads
                for X in "AB":
                    for chk in range(MC):
                        sl = slice(512 * chk, 512 * (chk + 1))
                        nc.tensor.matmul(
                            pss[X][:, sl], ident_sb[:], mbp_sb[:, qt, sl],
                            start=False, stop=(chk == MC - 1),
                            skip_group_check=True)
                for X in "AB":
                    nc.vector.tensor_reduce(
                        negmax[X][:, qt:qt + 1], pss[X][:],
                        axis=mybir.AxisListType.X, op=ALU.max, negate=True)
                    if qt % 2 == 0:
                        pn2[X] = pn_pool.tile([128, 2, G], BF16, tag="pn",
                                              name=f"pn{X}{i}{qt}")
                    # P = exp(S + M - max), unnormalized, straight into the
                    # transpose slab; rowsum accumulated for host normalize
                    nc.scalar.activation(
                        pn2[X][:, qt % 2], pss[X][:], AF.Exp,
                        bias=negmax[X][:, qt:qt + 1], scale=1.0,
                        accum_out=rowsum[X][:, qt:qt + 1])
                    if qt % 2 == 1:
                        nc.sync.dma_start_transpose(
                            out=PT[X][:, qt // 2], in_=pn2[X][:])
                # AV for slab s once its transpose is ~2 slabs old
                if qt % 2 == 1 and qt >= 3:
                    emit_av_slab(qt // 2 - 1)

            # next pair's projections fill the PE bubble while softmax drains
            if i + 2 < len(pairs):
                emit_proj(i + 2)
            emit_av_slab(Q2 - 1)

            nc.gpsimd.dma_start(out=rs_ext[b, hA], in_=rowsum["A"][:])
            nc.gpsimd.dma_start(out=rs_ext[b, hB], in_=rowsum["B"][:])
            if hp == NH // 2 - 1 and b + 1 < B:
                emit_v_batch(b + 1)

    nc.compile()
    return nc


# ---------------------------------------------------------------------------
# Host-side wrapper: shard over batch across 8 cores, run SPMD, gather.
# ---------------------------------------------------------------------------
import numpy as np
import ml_dtypes

N_CORES = 8
_B_FULL, _NH, _G, _I, _D = 16, 8, 1024, 256, 64
_B_PER_CORE = _B_FULL // N_CORES
_KT = _I // 128

_cached_nc = None


def _get_nc():
    global _cached_nc
    if _cached_nc is None:
        _cached_nc = build_attention(B=_B_PER_CORE, NH=_NH, G=_G, I=_I, D=_D)
    return _cached_nc


def _split_bf16(x):
    hi = x.astype(ml_dtypes.bfloat16)
    lo = (x - hi.astype(np.float32)).astype(ml_dtypes.bfloat16)
    return hi, lo


def _make_in_maps(h, mask, W_Q, W_K, W_V):
    hT = np.ascontiguousarray(np.transpose(np.asarray(h, np.float32), (0, 2, 1)))
    hTb, hTl = _split_bf16(hT)
    wq = np.asarray(W_Q, np.float32) / np.sqrt(np.float32(_D))
    wk = np.asarray(W_K, np.float32)

    def _pack_w(w):                                         # [NH, I, D] -> [I, NH*D]
        return np.ascontiguousarray(
            w.transpose(1, 0, 2).reshape(_I, _NH * _D))

    wqh, wql = _split_bf16(_pack_w(wq))
    wkh, wkl = _split_bf16(_pack_w(wk))
    wv = np.asarray(W_V, np.float32)                       # [NH, I, D]
    wvp = np.ascontiguousarray(
        wv.transpose(1, 0, 2).reshape(_KT, 128, _NH * _D)
    ).astype(ml_dtypes.bfloat16)
    mbp = np.where(np.asarray(mask) != 0, np.float32(-1e30),
                   np.float32(0.0)).astype(ml_dtypes.bfloat16)
    ident = np.eye(128).astype(ml_dtypes.bfloat16)
    identn = (-np.eye(128)).astype(np.float16)
    return [
        {
            "hTb": np.ascontiguousarray(hTb[c * _B_PER_CORE:(c + 1) * _B_PER_CORE]),
            "hTl": np.ascontiguousarray(hTl[c * _B_PER_CORE:(c + 1) * _B_PER_CORE]),
            "mbp": mbp,
            "wqh": np.ascontiguousarray(wqh), "wql": np.ascontiguousarray(wql),
            "wkh": np.ascontiguousarray(wkh), "wkl": np.ascontiguousarray(wkl),
            "wvp": wvp,
            "ident": ident, "identn": identn,
        }
        for c in range(N_CORES)
    ]


def kernel(h, mask, W_Q, W_K, W_V):
    """h [16,1024,256] f32, mask [1024,1024] i32, W_* [8,256,64] f32
    -> [16, 8, 1024, 64] f32"""
    from concourse.bass_utils import run_bass_kernel_spmd

    nc = _get_nc()
    in_maps = _make_in_maps(h, mask, W_Q, W_K, W_V)
    res = run_bass_kernel_spmd(nc, in_maps, core_ids=list(range(N_CORES)))
    _QT = _G // 128
    outs = [np.asarray(res.results[c]["out"]).reshape(
                _B_PER_CORE, _NH, 2, 128, _QT // 2, _D)
            for c in range(N_CORES)]
    rss = [np.asarray(res.results[c]["rs"]).reshape(_B_PER_CORE, _NH, 128,
                                                    _QT)
           for c in range(N_CORES)]
    # out[b, h, half, p, qt, d]: g = half*512 + qt*128 + p
    full = np.concatenate(outs, axis=0).transpose(0, 1, 2, 4, 3, 5).reshape(
        _B_FULL, _NH, _G, _D)
    # rs[b, h, p, qt]: g = qt*128 + p
    rs = np.concatenate(rss, axis=0).transpose(0, 1, 3, 2).reshape(
        _B_FULL, _NH, _G)
    out = full.astype(np.float64) / rs.astype(np.float64)[..., None]
    return np.ascontiguousarray(out.astype(np.float32))


# revision 54
# speedup vs baseline: 1.1068x; 1.1068x over previous
"""Sparse-attention SPMD kernel (one NeuronCore program), v7.

Per core: B=2 batches x NH=8 heads as 4*B head-pairs (A, B).

PE model (measured): strictly one moving stream at a time, 1 col/cycle
@2.4GHz warm; LDWEIGHTS hides behind the previous stream. Optimization
= minimize total streamed columns:
  - projections: A/B heads share the SAME moving stream (hx chunk), so
    they are fused into one 128-row matmul (half the streams of v5)
  - S = Q.K^T: 3 bf16 passes (hi*hi + hi*lo + lo*hi ~= 17-bit), A/B are
    row-split 64-contraction streams (cannot fuse: different data)
  - mask add: identity matmul accumulates -1e30 bias into S psum
  - softmax: DVE negmax reduce; ACT exp (bias=negmax) writes bf16 P
    directly into the transpose slab, accum rowsum; NO normalize on
    device - host divides by rowsum (saves DVE mul+recip)
  - transpose: DMA-xbar per 2-qt slab [128, 2048] on sync HWDGE ring
  - AV: P^T is the STATIONARY operand (PT slab [m,q] tiles as weights),
    V streams (64 cols/step instead of 512): AV halves to 4096
    stream-cols per (b,h). Output lands q-major [128(q), qt, d] in one
    psum bank per half-pair, DMA'd straight to DRAM [B, NH, G, D].
Host: out / rowsum (float64 divide), no transpose needed.
"""
import sys

sys.path.insert(0, '/opt/trn_rl_repo')
from contextlib import ExitStack

import concourse.bass as bass
import concourse.tile as tile
from concourse import bacc, mybir

FP32 = mybir.dt.float32
BF16 = mybir.dt.bfloat16
FP16 = mybir.dt.float16
F8E4 = mybir.dt.float8e4
AF = mybir.ActivationFunctionType
ALU = mybir.AluOpType
PM = mybir.MatmulPerfMode


def build_attention(B=2, NH=8, G=1024, I=256, D=64, tp_batch=2):
    assert D == 64 and I % 128 == 0 and G % 512 == 0 and NH % 2 == 0
    KT = I // 128          # contraction k-tiles for projections
    QT = G // 128          # q tiles
    MC = G // 512          # m chunks of 512 (S rhs / psum bank)
    MT = G // 128          # m tiles
    Q2 = QT // 2           # 2-qt transpose slabs

    nc = bacc.Bacc(None, target_bir_lowering=False, debug=False)
    dram = {}
    for nm, shp, dt in [
        ("hTb", [B, I, G], BF16), ("hTl", [B, I, G], BF16),
        ("mbp", [G, G], BF16),
        ("wqh", [I, NH * D], BF16), ("wql", [I, NH * D], BF16),
        ("wkh", [I, NH * D], BF16), ("wkl", [I, NH * D], BF16),
        ("wvp", [KT, 128, NH * D], BF16),
        ("ident", [128, 128], BF16), ("identn", [128, 128], FP16),
    ]:
        dram[nm] = nc.declare_dram_parameter(nm, shp, dt, isOutput=False)
    # device-natural layouts (contiguous per partition; host reorders):
    # out[b, h, half, p, qt, d] with g = half*512 + qt*128 + p
    out_ext = nc.declare_dram_parameter("out", [B, NH, 2, 128, QT // 2, D],
                                        FP32, isOutput=True)
    # rs[b, h, p, qt] with g = qt*128 + p
    rs_ext = nc.declare_dram_parameter("rs", [B, NH, 128, QT], FP32,
                                       isOutput=True)

    ctx = ExitStack()
    with ctx:
        tc = ctx.enter_context(tile.TileContext(nc))
        const = ctx.enter_context(tc.tile_pool(name="const", bufs=1))
        vpool = ctx.enter_context(tc.tile_pool(name="vsb", bufs=1))
        qk_pool = ctx.enter_context(tc.tile_pool(name="qk", bufs=2))
        f8_pool = ctx.enter_context(tc.tile_pool(name="f8", bufs=2))
        pn_pool = ctx.enter_context(tc.tile_pool(name="pn", bufs=6))
        pt_pool = ctx.enter_context(tc.tile_pool(name="pt", bufs=4))
        o_pool = ctx.enter_context(tc.tile_pool(name="o", bufs=4))
        st_pool = ctx.enter_context(tc.tile_pool(name="stats", bufs=4))
        # PSUM budget (8 banks): pss 3x2 + misc 2x1 = 8. The misc pool is
        # shared by proj / V-proj / AV psums (temporally mostly disjoint;
        # a single tag rotates the two banks).
        ps_s = ctx.enter_context(tc.tile_pool(name="pss", bufs=3,
                                              space="PSUM"))
        ps_misc = ctx.enter_context(tc.tile_pool(name="psmisc", bufs=2,
                                                 space="PSUM"))

        # ---------- setup: load inputs (gpsimd SWDGE ring; sync ring is
        # reserved for the xbar transposes) ----------
        # DMA issue order matters: proj(0) needs hT(b0) + wq + identn first;
        # mask (mbp/ident) is only needed ~15us in.
        hTb_sb = const.tile([128, B, KT, G], BF16)
        hTl_sb = const.tile([128, B, KT, G], BF16)
        w_sb = {}
        for nm in ("wqh", "wql", "wkh", "wkl"):
            # [128(i), kt, h, d] heads contiguous within a kt (packed on
            # host): the head-pair weight slice [:, kt, 2hp:2hp+2, :]
            # merges to a single 128-elem free dim
            w_sb[nm] = const.tile([128, KT, NH, D], BF16, name=nm)
        wv_sb = const.tile([128, KT, NH * D], BF16)
        ident_sb = const.tile([128, 128], BF16)
        identn_sb = const.tile([128, 128], FP16)
        mbp_sb = const.tile([128, QT, G], BF16)

        def _load_w(nm):
            nc.gpsimd.dma_start(
                out=w_sb[nm][:],
                in_=dram[nm].rearrange("(kt p) (h d) -> p kt h d",
                                       p=128, d=D))

        def _load_h(nm, sb, b):
            nc.gpsimd.dma_start(
                out=sb[:, b],
                in_=dram[nm][b].rearrange("(kt p) g -> p kt g", p=128))

        def _load_h_kt(nm, sb, b, kt):
            nc.gpsimd.dma_start(
                out=sb[:, b, kt],
                in_=dram[nm][b, 128 * kt:128 * (kt + 1), :])

        _load_w("wqh")
        _load_h_kt("hTb", hTb_sb, 0, 0)
        _load_w("wql")
        _load_h_kt("hTl", hTl_sb, 0, 0)
        _load_h_kt("hTb", hTb_sb, 0, 1)
        _load_h_kt("hTl", hTl_sb, 0, 1)
        nc.gpsimd.dma_start(out=identn_sb[:], in_=dram["identn"][:])
        _load_w("wkh")
        _load_w("wkl")
        nc.gpsimd.dma_start(out=wv_sb[:],
                            in_=dram["wvp"].rearrange("kt p hd -> p kt hd"))
        nc.gpsimd.dma_start(out=ident_sb[:], in_=dram["ident"][:])
        nc.gpsimd.dma_start(
            out=mbp_sb[:],
            in_=dram["mbp"].rearrange("(qt p) m -> p qt m", p=128))
        for b in range(1, B):
            _load_h("hTb", hTb_sb, b)
            _load_h("hTl", hTl_sb, b)

        # V for both batches: [128(m), b, mt, NH*D] bf16
        v_sb = vpool.tile([128, B, MT, NH * D], BF16)

        def emit_v_batch(b):
            for mt in range(MT):
                psv = ps_misc.tile([128, 512], FP32, tag="m",
                                   name=f"psv{b}{mt}")
                for kt in range(KT):
                    nc.tensor.matmul(
                        psv[:], hTb_sb[:, b, kt, 128 * mt:128 * (mt + 1)],
                        wv_sb[:, kt], start=(kt == 0), stop=(kt == KT - 1))
                if mt % 2 == 0:
                    nc.scalar.copy(v_sb[:, b, mt], psv[:])
                else:
                    nc.vector.tensor_copy(v_sb[:, b, mt], psv[:])

        # ---------- main loop over (batch, head-pair), software-pipelined ----
        pairs = [(b, hp) for b in range(B) for hp in range(NH // 2)]
        state = {}

        def emit_proj(i):
            """Q^T/K^T for pair i: 3-pass bf16 projections; A/B heads
            fused into one 128-row matmul per (chunk, kt, pass) since
            they share the moving stream. psum group is then split into
            fp16 hi/lo tiles (ACT copy, -I matmul, DVE copy), and
            scale-balanced fp8e4 copies are derived on gpsimd for the
            DoubleRow S cross pass:
              f8q planes (W): qh*2^-4, ql*2^7; f8k planes (X): kl*2^4,
              kh*2^-7 -> DR computes qh*kl + ql*kh with product scale 1.
            """
            b, hp = pairs[i]
            qk_hi = qk_pool.tile([128, 2 * G], FP16, tag="qkh", name=f"qkh{i}")
            qk_lo = qk_pool.tile([128, 2 * G], FP16, tag="qkl", name=f"qkl{i}")
            f8q = f8_pool.tile([128, 2, G], F8E4, tag="f8q", name=f"f8q{i}")
            f8k = f8_pool.tile([128, 2, G], F8E4, tag="f8k", name=f"f8k{i}")
            for W, off in (("q", 0), ("k", G)):
                wh, wl = w_sb["w" + W + "h"], w_sb["w" + W + "l"]
                for chk in range(G // 512):
                    sl = slice(512 * chk, 512 * (chk + 1))
                    psp = ps_misc.tile([128, 512], FP32, tag="m",
                                       name=f"psp{i}{W}{chk}")
                    first = True
                    for kt in range(KT):
                        for wx, hx in ((wh, hTb_sb), (wh, hTl_sb),
                                       (wl, hTb_sb)):
                            # [128, 2, 64]->[128, 128] weight AP (merged)
                            nc.tensor.matmul(
                                psp[:],
                                wx[:, kt, 2 * hp:2 * hp + 2, :],
                                hx[:, b, kt, sl],
                                start=first, stop=False,
                                skip_group_check=True)
                            first = False
                    csl = slice(off + 512 * chk, off + 512 * (chk + 1))
                    nc.scalar.copy(qk_hi[:, csl], psp[:])
                    # psp -= hi  (exact residual), then lo = fp16(psp)
                    nc.tensor.matmul(psp[:], identn_sb[:], qk_hi[:, csl],
                                     start=False, stop=True,
                                     skip_group_check=True)
                    nc.vector.tensor_copy(qk_lo[:, csl], psp[:])
                    if W == "q":
                        nc.scalar.activation(f8q[:, 0, sl], qk_hi[:, csl],
                                             AF.Copy, scale=2.0 ** -4)
                        nc.vector.tensor_scalar_mul(
                            f8q[:, 1, sl], qk_lo[:, csl], 2.0 ** 7)
                    else:
                        nc.vector.tensor_scalar_mul(
                            f8k[:, 0, sl], qk_lo[:, csl], 2.0 ** 4)
                        nc.scalar.activation(f8k[:, 1, sl], qk_hi[:, csl],
                                             AF.Copy, scale=2.0 ** -7)
            state[i] = (qk_hi, qk_lo, f8q, f8k)

        rows_of = {"A": slice(0, 64), "B": slice(64, 128)}

        emit_proj(0)
        emit_v_batch(0)
        emit_proj(1)
        for i, (b, hp) in enumerate(pairs):
            hA, hB = 2 * hp, 2 * hp + 1
            qk_hi, qk_lo, f8q, f8k = state.pop(i)

            negmax = {X: st_pool.tile([128, QT], FP32, tag=f"negmax{X}",
                                      name=f"negmax{X}{i}") for X in "AB"}
            rowsum = {X: st_pool.tile([128, QT], FP32, tag=f"rowsum{X}",
                                      name=f"rowsum{X}{i}") for X in "AB"}
            # PT layout: [m_in(128), qt2, qt_lo(2), mt(8), q_in(128)]
            PT = {X: pt_pool.tile([128, Q2, 2, MT, 128], BF16, tag="pt",
                                  name=f"PT{X}{i}") for X in "AB"}
            pn2 = {}

            av_ps = {}

            def emit_av_slab(s):
                """AV for slab s (qts 2s, 2s+1): PT tiles as weights, V
                streams 64 cols; accumulate over mt into one psum bank
                per half-pair; after an odd slab, SBUF-stage + DMA out."""
                half = s // 2
                if s % 2 == 0:
                    av_ps[half] = ps_misc.tile([128, QT // 2, 128], FP32,
                                               tag="m", name=f"av{i}{half}")
                av = av_ps[half]
                for l in range(2):
                    qt = 2 * s + l
                    lq = qt % (QT // 2)
                    for X, hh in (("A", hA), ("B", hB)):
                        xo = 0 if X == "A" else 64
                        for mt in range(MT):
                            nc.tensor.matmul(
                                av[:, lq, xo:xo + 64],
                                PT[X][:, s, l, mt, :],
                                v_sb[:, b, mt, 64 * hh:64 * (hh + 1)],
                                start=(mt == 0), stop=(mt == MT - 1),
                                skip_group_check=True)
                if s % 2 == 1:
                    o_sb = o_pool.tile([128, QT // 2, 128], FP32, tag="o",
                                       name=f"o{i}{half}")
                    if half == 0:
                        nc.scalar.copy(o_sb[:], av[:])
                    else:
                        nc.vector.tensor_copy(o_sb[:], av[:])
                    for X, hh in (("A", hA), ("B", hB)):
                        xo = 0 if X == "A" else 64
                        nc.gpsimd.dma_start(
                            out=out_ext[b, hh, half],
                            in_=o_sb[:, :, xo:xo + 64])

            for qt in range(QT):
                pss = {X: ps_s.tile([128, G], FP32, tag="s",
                                    name=f"pss{X}{i}{qt}") for X in "AB"}
                # S: 3 bf16 passes per (qt, chunk, X); A/B row groups
                qsl = slice(128 * qt, 128 * (qt + 1))
                for chk in range(MC):
                    sl = slice(512 * chk, 512 * (chk + 1))
                    ksl = slice(G + 512 * chk, G + 512 * (chk + 1))
                    for X in "AB":
                        rows = rows_of[X]
                        # pass 1: hi*hi in fp16
                        nc.tensor.matmul(
                            pss[X][:, sl], qk_hi[rows, qsl],
                            qk_hi[rows, ksl],
                            start=True, stop=False, skip_group_check=True)
                        # pass 2: both cross terms in one fp8 DoubleRow
                        nc.tensor.matmul(
                            pss[X][:, sl], f8q[rows, :, qsl],
                            f8k[rows, :, sl],
                            perf_mode=PM.DoubleRow,
                            start=False, stop=False, skip_group_check=True)
                # mask add via identity matmul (PE), both heads
                for X in "AB":
                    for chk in range(MC):
                        sl = slice(512 * chk, 512 * (chk + 1))
                        nc.tensor.matmul(
                            pss[X][:, sl], ident_sb[:], mbp_sb[:, qt, sl],
                            start=False, stop=(chk == MC - 1),
                            skip_group_check=True)
                for X in "AB":
                    nc.vector.tensor_reduce(
                        negmax[X][:, qt:qt + 1], pss[X][:],
                        axis=mybir.AxisListType.X, op=ALU.max, negate=True)
                    if qt % 2 == 0:
                        pn2[X] = pn_pool.tile([128, 2, G], BF16, tag="pn",
                                              name=f"pn{X}{i}{qt}")
                    # P = exp(S + M - max), unnormalized, straight into the
                    # transpose slab; rowsum accumulated for host normalize
                    nc.scalar.activation(
                        pn2[X][:, qt % 2], pss[X][:], AF.Exp,
                        bias=negmax[X][:, qt:qt + 1], scale=1.0,
                        accum_out=rowsum[X][:, qt:qt + 1])
                    if qt % 2 == 1:
                        nc.sync.dma_start_transpose(
                            out=PT[X][:, qt // 2], in_=pn2[X][:])

            # AV(half0): its transposes are ~4 qt old, runs stall-free;
            # proj(i+2) then fills the PE while slab3's transpose lands.
            emit_av_slab(0)
            emit_av_slab(1)
            if i + 2 < len(pairs):
                emit_proj(i + 2)
            emit_av_slab(2)
            emit_av_slab(Q2 - 1)

            nc.gpsimd.dma_start(out=rs_ext[b, hA], in_=rowsum["A"][:])
            nc.gpsimd.dma_start(out=rs_ext[b, hB], in_=rowsum["B"][:])
            if hp == NH // 2 - 1 and b + 1 < B:
                emit_v_batch(b + 1)

    nc.compile()
    return nc


# ---------------------------------------------------------------------------
# Host-side wrapper: shard over batch across 8 cores, run SPMD, gather.
# ---------------------------------------------------------------------------
import numpy as np
import ml_dtypes

N_CORES = 8
_B_FULL, _NH, _G, _I, _D = 16, 8, 1024, 256, 64
_B_PER_CORE = _B_FULL // N_CORES
_KT = _I // 128

_cached_nc = None


def _get_nc():
    global _cached_nc
    if _cached_nc is None:
        _cached_nc = build_attention(B=_B_PER_CORE, NH=_NH, G=_G, I=_I, D=_D)
    return _cached_nc


def _split_bf16(x):
    hi = x.astype(ml_dtypes.bfloat16)
    lo = (x - hi.astype(np.float32)).astype(ml_dtypes.bfloat16)
    return hi, lo


def _make_in_maps(h, mask, W_Q, W_K, W_V):
    hT = np.ascontiguousarray(np.transpose(np.asarray(h, np.float32), (0, 2, 1)))
    hTb, hTl = _split_bf16(hT)
    wq = np.asarray(W_Q, np.float32) / np.sqrt(np.float32(_D))
    wk = np.asarray(W_K, np.float32)

    def _pack_w(w):                                         # [NH, I, D] -> [I, NH*D]
        return np.ascontiguousarray(
            w.transpose(1, 0, 2).reshape(_I, _NH * _D))

    wqh, wql = _split_bf16(_pack_w(wq))
    wkh, wkl = _split_bf16(_pack_w(wk))
    wv = np.asarray(W_V, np.float32)                       # [NH, I, D]
    wvp = np.ascontiguousarray(
        wv.transpose(1, 0, 2).reshape(_KT, 128, _NH * _D)
    ).astype(ml_dtypes.bfloat16)
    mbp = np.where(np.asarray(mask) != 0, np.float32(-1e30),
                   np.float32(0.0)).astype(ml_dtypes.bfloat16)
    ident = np.eye(128).astype(ml_dtypes.bfloat16)
    identn = (-np.eye(128)).astype(np.float16)
    return [
        {
            "hTb": np.ascontiguousarray(hTb[c * _B_PER_CORE:(c + 1) * _B_PER_CORE]),
            "hTl": np.ascontiguousarray(hTl[c * _B_PER_CORE:(c + 1) * _B_PER_CORE]),
            "mbp": mbp,
            "wqh": np.ascontiguousarray(wqh), "wql": np.ascontiguousarray(wql),
            "wkh": np.ascontiguousarray(wkh), "wkl": np.ascontiguousarray(wkl),
            "wvp": wvp,
            "ident": ident, "identn": identn,
        }
        for c in range(N_CORES)
    ]


def kernel(h, mask, W_Q, W_K, W_V):
    """h [16,1024,256] f32, mask [1024,1024] i32, W_* [8,256,64] f32
    -> [16, 8, 1024, 64] f32"""
    from concourse.bass_utils import run_bass_kernel_spmd

    nc = _get_nc()
    in_maps = _make_in_maps(h, mask, W_Q, W_K, W_V)
    res = run_bass_kernel_spmd(nc, in_maps, core_ids=list(range(N_CORES)))
    _QT = _G // 128
    outs = [np.asarray(res.results[c]["out"]).reshape(
                _B_PER_CORE, _NH, 2, 128, _QT // 2, _D)
            for c in range(N_CORES)]
    rss = [np.asarray(res.results[c]["rs"]).reshape(_B_PER_CORE, _NH, 128,
                                                    _QT)
           for c in range(N_CORES)]
    # out[b, h, half, p, qt, d]: g = half*512 + qt*128 + p
    full = np.concatenate(outs, axis=0).transpose(0, 1, 2, 4, 3, 5).reshape(
        _B_FULL, _NH, _G, _D)
    # rs[b, h, p, qt]: g = qt*128 + p
    rs = np.concatenate(rss, axis=0).transpose(0, 1, 3, 2).reshape(
        _B_FULL, _NH, _G)
    out = full.astype(np.float64) / rs.astype(np.float64)[..., None]
    return np.ascontiguousarray(out.astype(np.float32))


# revision 55
# speedup vs baseline: 1.2745x; 1.1515x over previous
"""Sparse-attention SPMD kernel (one NeuronCore program), v7.

Per core: B=2 batches x NH=8 heads as 4*B head-pairs (A, B).

PE model (measured): strictly one moving stream at a time, 1 col/cycle
@2.4GHz warm; LDWEIGHTS hides behind the previous stream. Optimization
= minimize total streamed columns:
  - projections: A/B heads share the SAME moving stream (hx chunk), so
    they are fused into one 128-row matmul (half the streams of v5)
  - S = Q.K^T: 3 bf16 passes (hi*hi + hi*lo + lo*hi ~= 17-bit), A/B are
    row-split 64-contraction streams (cannot fuse: different data)
  - mask add: identity matmul accumulates -1e30 bias into S psum
  - softmax: DVE negmax reduce; ACT exp (bias=negmax) writes bf16 P
    directly into the transpose slab, accum rowsum; NO normalize on
    device - host divides by rowsum (saves DVE mul+recip)
  - transpose: DMA-xbar per 2-qt slab [128, 2048] on sync HWDGE ring
  - AV: P^T is the STATIONARY operand (PT slab [m,q] tiles as weights),
    V streams (64 cols/step instead of 512): AV halves to 4096
    stream-cols per (b,h). Output lands q-major [128(q), qt, d] in one
    psum bank per half-pair, DMA'd straight to DRAM [B, NH, G, D].
Host: out / rowsum (float64 divide), no transpose needed.
"""
import sys

sys.path.insert(0, '/opt/trn_rl_repo')
from contextlib import ExitStack

import concourse.bass as bass
import concourse.tile as tile
from concourse import bacc, mybir

FP32 = mybir.dt.float32
BF16 = mybir.dt.bfloat16
FP16 = mybir.dt.float16
F8E4 = mybir.dt.float8e4
AF = mybir.ActivationFunctionType
ALU = mybir.AluOpType
PM = mybir.MatmulPerfMode


def build_attention(B=2, NH=8, G=1024, I=256, D=64, tp_batch=2):
    assert D == 64 and I % 128 == 0 and G % 512 == 0 and NH % 2 == 0
    KT = I // 128          # contraction k-tiles for projections
    QT = G // 128          # q tiles
    MC = G // 512          # m chunks of 512 (S rhs / psum bank)
    MT = G // 128          # m tiles
    Q2 = QT // 2           # 2-qt transpose slabs

    nc = bacc.Bacc(None, target_bir_lowering=False, debug=False)
    dram = {}
    for nm, shp, dt in [
        ("hTb", [B, I, G], BF16), ("hTl", [B, I, G], BF16),
        ("mbp", [G, G], BF16),
        ("wqh", [I, NH * D], BF16), ("wql", [I, NH * D], BF16),
        ("wkh", [I, NH * D], BF16), ("wkl", [I, NH * D], BF16),
        ("wvp", [KT, 128, NH * D], BF16),
        ("ident", [128, 128], BF16), ("identn", [128, 128], FP16),
    ]:
        dram[nm] = nc.declare_dram_parameter(nm, shp, dt, isOutput=False)
    # device-natural layouts (contiguous per partition; host reorders):
    # out[b, h, half, p, qt, d] with g = half*512 + qt*128 + p
    out_ext = nc.declare_dram_parameter("out", [B, NH, 2, 128, QT // 2, D],
                                        FP32, isOutput=True)
    # rs[b, h, p, qt] with g = qt*128 + p
    rs_ext = nc.declare_dram_parameter("rs", [B, NH, 128, QT], FP32,
                                       isOutput=True)

    ctx = ExitStack()
    with ctx:
        tc = ctx.enter_context(tile.TileContext(nc))
        const = ctx.enter_context(tc.tile_pool(name="const", bufs=1))
        vpool = ctx.enter_context(tc.tile_pool(name="vsb", bufs=1))
        qk_pool = ctx.enter_context(tc.tile_pool(name="qk", bufs=2))
        f8_pool = ctx.enter_context(tc.tile_pool(name="f8", bufs=2))
        pn_pool = ctx.enter_context(tc.tile_pool(name="pn", bufs=6))
        pt_pool = ctx.enter_context(tc.tile_pool(name="pt", bufs=4))
        o_pool = ctx.enter_context(tc.tile_pool(name="o", bufs=4))
        st_pool = ctx.enter_context(tc.tile_pool(name="stats", bufs=4))
        # PSUM budget (8 banks): pss 3x2 + misc 2x1 = 8. The misc pool is
        # shared by proj / V-proj / AV psums (temporally mostly disjoint;
        # a single tag rotates the two banks).
        ps_s = ctx.enter_context(tc.tile_pool(name="pss", bufs=3,
                                              space="PSUM"))
        ps_misc = ctx.enter_context(tc.tile_pool(name="psmisc", bufs=2,
                                                 space="PSUM"))

        # ---------- setup: load inputs (gpsimd SWDGE ring; sync ring is
        # reserved for the xbar transposes) ----------
        # DMA issue order matters: proj(0) needs hT(b0) + wq + identn first;
        # mask (mbp/ident) is only needed ~15us in.
        hTb_sb = const.tile([128, B, KT, G], BF16)
        hTl_sb = const.tile([128, B, KT, G], BF16)
        w_sb = {}
        for nm in ("wqh", "wql", "wkh", "wkl"):
            # [128(i), kt, h, d] heads contiguous within a kt (packed on
            # host): the head-pair weight slice [:, kt, 2hp:2hp+2, :]
            # merges to a single 128-elem free dim
            w_sb[nm] = const.tile([128, KT, NH, D], BF16, name=nm)
        wv_sb = const.tile([128, KT, NH * D], BF16)
        ident_sb = const.tile([128, 128], BF16)
        identn_sb = const.tile([128, 128], FP16)
        mbp_sb = const.tile([128, QT, G], BF16)

        def _load_w(nm):
            nc.gpsimd.dma_start(
                out=w_sb[nm][:],
                in_=dram[nm].rearrange("(kt p) (h d) -> p kt h d",
                                       p=128, d=D))

        def _load_h(nm, sb, b):
            nc.gpsimd.dma_start(
                out=sb[:, b],
                in_=dram[nm][b].rearrange("(kt p) g -> p kt g", p=128))

        def _load_h_kt(nm, sb, b, kt):
            nc.gpsimd.dma_start(
                out=sb[:, b, kt],
                in_=dram[nm][b, 128 * kt:128 * (kt + 1), :])

        _load_w("wqh")
        _load_h_kt("hTb", hTb_sb, 0, 0)
        _load_w("wql")
        _load_h_kt("hTl", hTl_sb, 0, 0)
        _load_h_kt("hTb", hTb_sb, 0, 1)
        _load_h_kt("hTl", hTl_sb, 0, 1)
        nc.gpsimd.dma_start(out=identn_sb[:], in_=dram["identn"][:])
        _load_w("wkh")
        _load_w("wkl")
        nc.gpsimd.dma_start(out=wv_sb[:],
                            in_=dram["wvp"].rearrange("kt p hd -> p kt hd"))
        nc.gpsimd.dma_start(out=ident_sb[:], in_=dram["ident"][:])
        nc.gpsimd.dma_start(
            out=mbp_sb[:],
            in_=dram["mbp"].rearrange("(qt p) m -> p qt m", p=128))
        for b in range(1, B):
            _load_h("hTb", hTb_sb, b)
            _load_h("hTl", hTl_sb, b)

        # V for both batches: [128(m), b, mt, NH*D] bf16
        v_sb = vpool.tile([128, B, MT, NH * D], BF16)

        def emit_v_batch(b):
            for mt in range(MT):
                psv = ps_misc.tile([128, 512], FP32, tag="m",
                                   name=f"psv{b}{mt}")
                for kt in range(KT):
                    nc.tensor.matmul(
                        psv[:], hTb_sb[:, b, kt, 128 * mt:128 * (mt + 1)],
                        wv_sb[:, kt], start=(kt == 0), stop=(kt == KT - 1))
                if mt % 2 == 0:
                    nc.scalar.copy(v_sb[:, b, mt], psv[:])
                else:
                    nc.vector.tensor_copy(v_sb[:, b, mt], psv[:])

        # ---------- main loop over (batch, head-pair), software-pipelined ----
        pairs = [(b, hp) for b in range(B) for hp in range(NH // 2)]
        state = {}

        def emit_proj(i):
            """Q^T/K^T for pair i: 3-pass bf16 projections; A/B heads
            fused into one 128-row matmul per (chunk, kt, pass) since
            they share the moving stream. psum group is then split into
            fp16 hi/lo tiles (ACT copy, -I matmul, DVE copy), and
            scale-balanced fp8e4 copies are derived on gpsimd for the
            DoubleRow S cross pass:
              f8q planes (W): qh*2^-4, ql*2^7; f8k planes (X): kl*2^4,
              kh*2^-7 -> DR computes qh*kl + ql*kh with product scale 1.
            """
            b, hp = pairs[i]
            qk_hi = qk_pool.tile([128, 2 * G], FP16, tag="qkh", name=f"qkh{i}")
            qk_lo = qk_pool.tile([128, 2 * G], FP16, tag="qkl", name=f"qkl{i}")
            f8q = f8_pool.tile([128, 2, G], F8E4, tag="f8q", name=f"f8q{i}")
            f8k = f8_pool.tile([128, 2, G], F8E4, tag="f8k", name=f"f8k{i}")
            for W, off in (("q", 0), ("k", G)):
                wh, wl = w_sb["w" + W + "h"], w_sb["w" + W + "l"]
                for chk in range(G // 512):
                    sl = slice(512 * chk, 512 * (chk + 1))
                    psp = ps_misc.tile([128, 512], FP32, tag="m",
                                       name=f"psp{i}{W}{chk}")
                    first = True
                    for kt in range(KT):
                        for wx, hx in ((wh, hTb_sb), (wh, hTl_sb),
                                       (wl, hTb_sb)):
                            # [128, 2, 64]->[128, 128] weight AP (merged)
                            nc.tensor.matmul(
                                psp[:],
                                wx[:, kt, 2 * hp:2 * hp + 2, :],
                                hx[:, b, kt, sl],
                                start=first, stop=False,
                                skip_group_check=True)
                            first = False
                    csl = slice(off + 512 * chk, off + 512 * (chk + 1))
                    nc.scalar.copy(qk_hi[:, csl], psp[:])
                    # psp -= hi  (exact residual), then lo = fp16(psp)
                    nc.tensor.matmul(psp[:], identn_sb[:], qk_hi[:, csl],
                                     start=False, stop=True,
                                     skip_group_check=True)
                    nc.vector.tensor_copy(qk_lo[:, csl], psp[:])
                    if W == "q":
                        nc.scalar.activation(f8q[:, 0, sl], qk_hi[:, csl],
                                             AF.Copy, scale=2.0 ** -4)
                        nc.vector.tensor_scalar_mul(
                            f8q[:, 1, sl], qk_lo[:, csl], 2.0 ** 7)
                    else:
                        nc.vector.tensor_scalar_mul(
                            f8k[:, 0, sl], qk_lo[:, csl], 2.0 ** 4)
                        nc.scalar.activation(f8k[:, 1, sl], qk_hi[:, csl],
                                             AF.Copy, scale=2.0 ** -7)
            state[i] = (qk_hi, qk_lo, f8q, f8k)

        rows_of = {"A": slice(0, 64), "B": slice(64, 128)}

        emit_proj(0)
        emit_v_batch(0)
        emit_proj(1)
        for i, (b, hp) in enumerate(pairs):
            hA, hB = 2 * hp, 2 * hp + 1
            qk_hi, qk_lo, f8q, f8k = state.pop(i)

            negmax = {X: st_pool.tile([128, QT], FP32, tag=f"negmax{X}",
                                      name=f"negmax{X}{i}") for X in "AB"}
            rowsum = {X: st_pool.tile([128, QT], FP32, tag=f"rowsum{X}",
                                      name=f"rowsum{X}{i}") for X in "AB"}
            # PT layout: [m_in(128), qt2, qt_lo(2), mt(8), q_in(128)]
            PT = {X: pt_pool.tile([128, Q2, 2, MT, 128], BF16, tag="pt",
                                  name=f"PT{X}{i}") for X in "AB"}
            pn2 = {}

            av_ps = {}

            def emit_av_slab(s):
                """AV for slab s (qts 2s, 2s+1): PT tiles as weights, V
                streams 64 cols; accumulate over mt into one psum bank
                per half-pair; after an odd slab, SBUF-stage + DMA out."""
                half = s // 2
                if s % 2 == 0:
                    av_ps[half] = ps_misc.tile([128, QT // 2, 128], FP32,
                                               tag="m", name=f"av{i}{half}")
                av = av_ps[half]
                for l in range(2):
                    qt = 2 * s + l
                    lq = qt % (QT // 2)
                    for X, hh in (("A", hA), ("B", hB)):
                        xo = 0 if X == "A" else 64
                        for mt in range(MT):
                            nc.tensor.matmul(
                                av[:, lq, xo:xo + 64],
                                PT[X][:, s, l, mt, :],
                                v_sb[:, b, mt, 64 * hh:64 * (hh + 1)],
                                start=(mt == 0), stop=(mt == MT - 1),
                                skip_group_check=True)
                if s % 2 == 1:
                    o_sb = o_pool.tile([128, QT // 2, 128], FP32, tag="o",
                                       name=f"o{i}{half}")
                    if half == 0:
                        nc.scalar.copy(o_sb[:], av[:])
                    else:
                        nc.vector.tensor_copy(o_sb[:], av[:])
                    for X, hh in (("A", hA), ("B", hB)):
                        xo = 0 if X == "A" else 64
                        nc.gpsimd.dma_start(
                            out=out_ext[b, hh, half],
                            in_=o_sb[:, :, xo:xo + 64])

            for qt in range(QT):
                pss = {X: ps_s.tile([128, G], FP32, tag="s",
                                    name=f"pss{X}{i}{qt}") for X in "AB"}
                # S: 3 bf16 passes per (qt, chunk, X); A/B row groups
                qsl = slice(128 * qt, 128 * (qt + 1))
                for chk in range(MC):
                    sl = slice(512 * chk, 512 * (chk + 1))
                    ksl = slice(G + 512 * chk, G + 512 * (chk + 1))
                    for X in "AB":
                        rows = rows_of[X]
                        # pass 1: hi*hi in fp16
                        nc.tensor.matmul(
                            pss[X][:, sl], qk_hi[rows, qsl],
                            qk_hi[rows, ksl],
                            start=True, stop=False, skip_group_check=True)
                        # pass 2: both cross terms in one fp8 DoubleRow
                        nc.tensor.matmul(
                            pss[X][:, sl], f8q[rows, :, qsl],
                            f8k[rows, :, sl],
                            perf_mode=PM.DoubleRow,
                            start=False, stop=False, skip_group_check=True)
                # mask add via identity matmul (PE), both heads
                for X in "AB":
                    for chk in range(MC):
                        sl = slice(512 * chk, 512 * (chk + 1))
                        nc.tensor.matmul(
                            pss[X][:, sl], ident_sb[:], mbp_sb[:, qt, sl],
                            start=False, stop=(chk == MC - 1),
                            skip_group_check=True)
                for X in "AB":
                    nc.vector.tensor_reduce(
                        negmax[X][:, qt:qt + 1], pss[X][:],
                        axis=mybir.AxisListType.X, op=ALU.max, negate=True)
                    if qt % 2 == 0:
                        pn2[X] = pn_pool.tile([128, 2, G], BF16, tag="pn",
                                              name=f"pn{X}{i}{qt}")
                    # P = exp(S + M - max), unnormalized, straight into the
                    # transpose slab; rowsum accumulated for host normalize
                    nc.scalar.activation(
                        pn2[X][:, qt % 2], pss[X][:], AF.Exp,
                        bias=negmax[X][:, qt:qt + 1], scale=1.0,
                        accum_out=rowsum[X][:, qt:qt + 1])
                    if qt % 2 == 1:
                        nc.sync.dma_start_transpose(
                            out=PT[X][:, qt // 2], in_=pn2[X][:])

            # next pair's projections fill the PE bubble while softmax drains
            if i + 2 < len(pairs):
                emit_proj(i + 2)
            for s in range(Q2):
                emit_av_slab(s)

            nc.gpsimd.dma_start(out=rs_ext[b, hA], in_=rowsum["A"][:])
            nc.gpsimd.dma_start(out=rs_ext[b, hB], in_=rowsum["B"][:])
            if hp == NH // 2 - 1 and b + 1 < B:
                emit_v_batch(b + 1)

    nc.compile()
    return nc


# ---------------------------------------------------------------------------
# Host-side wrapper: shard over batch across 8 cores, run SPMD, gather.
# ---------------------------------------------------------------------------
import numpy as np
import ml_dtypes

N_CORES = 8
_B_FULL, _NH, _G, _I, _D = 16, 8, 1024, 256, 64
_B_PER_CORE = _B_FULL // N_CORES
_KT = _I // 128

_cached_nc = None


def _get_nc():
    global _cached_nc
    if _cached_nc is None:
        _cached_nc = build_attention(B=_B_PER_CORE, NH=_NH, G=_G, I=_I, D=_D)
    return _cached_nc


def _split_bf16(x):
    hi = x.astype(ml_dtypes.bfloat16)
    lo = (x - hi.astype(np.float32)).astype(ml_dtypes.bfloat16)
    return hi, lo


def _make_in_maps(h, mask, W_Q, W_K, W_V):
    hT = np.ascontiguousarray(np.transpose(np.asarray(h, np.float32), (0, 2, 1)))
    hTb, hTl = _split_bf16(hT)
    wq = np.asarray(W_Q, np.float32) / np.sqrt(np.float32(_D))
    wk = np.asarray(W_K, np.float32)

    def _pack_w(w):                                         # [NH, I, D] -> [I, NH*D]
        return np.ascontiguousarray(
            w.transpose(1, 0, 2).reshape(_I, _NH * _D))

    wqh, wql = _split_bf16(_pack_w(wq))
    wkh, wkl = _split_bf16(_pack_w(wk))
    wv = np.asarray(W_V, np.float32)                       # [NH, I, D]
    wvp = np.ascontiguousarray(
        wv.transpose(1, 0, 2).reshape(_KT, 128, _NH * _D)
    ).astype(ml_dtypes.bfloat16)
    mbp = np.where(np.asarray(mask) != 0, np.float32(-1e30),
                   np.float32(0.0)).astype(ml_dtypes.bfloat16)
    ident = np.eye(128).astype(ml_dtypes.bfloat16)
    identn = (-np.eye(128)).astype(np.float16)
    return [
        {
            "hTb": np.ascontiguousarray(hTb[c * _B_PER_CORE:(c + 1) * _B_PER_CORE]),
            "hTl": np.ascontiguousarray(hTl[c * _B_PER_CORE:(c + 1) * _B_PER_CORE]),
            "mbp": mbp,
            "wqh": np.ascontiguousarray(wqh), "wql": np.ascontiguousarray(wql),
            "wkh": np.ascontiguousarray(wkh), "wkl": np.ascontiguousarray(wkl),
            "wvp": wvp,
            "ident": ident, "identn": identn,
        }
        for c in range(N_CORES)
    ]


def kernel(h, mask, W_Q, W_K, W_V):
    """h [16,1024,256] f32, mask [1024,1024] i32, W_* [8,256,64] f32
    -> [16, 8, 1024, 64] f32"""
    from concourse.bass_utils import run_bass_kernel_spmd

    nc = _get_nc()
    in_maps = _make_in_maps(h, mask, W_Q, W_K, W_V)
    res = run_bass_kernel_spmd(nc, in_maps, core_ids=list(range(N_CORES)))
    _QT = _G // 128
    outs = [np.asarray(res.results[c]["out"]).reshape(
                _B_PER_CORE, _NH, 2, 128, _QT // 2, _D)
            for c in range(N_CORES)]
    rss = [np.asarray(res.results[c]["rs"]).reshape(_B_PER_CORE, _NH, 128,
                                                    _QT)
           for c in range(N_CORES)]
    # out[b, h, half, p, qt, d]: g = half*512 + qt*128 + p
    full = np.concatenate(outs, axis=0).transpose(0, 1, 2, 4, 3, 5).reshape(
        _B_FULL, _NH, _G, _D)
    # rs[b, h, p, qt]: g = qt*128 + p
    rs = np.concatenate(rss, axis=0).transpose(0, 1, 3, 2).reshape(
        _B_FULL, _NH, _G)
    out = full.astype(np.float64) / rs.astype(np.float64)[..., None]
    return np.ascontiguousarray(out.astype(np.float32))
